# revision 71
# baseline (speedup 1.0000x reference)
"""Clustered Linformer Attention — Trainium2 Bass kernel, 8 NeuronCores.

Strategy: data-parallel over batch (2 batches/core, no collectives).
~142us HW exec (baseline 254us), rel err ~1.56e-2 (gate 2e-2, deterministic).
Math restructuring (exact vs reference up to rounding):
  - mask is all-ones => cluster c holds positions [32c, 32c+32); K/V are only
    consumed through the cluster projections, so reassociate:
      k_proj_h = (M_Eh x) Wk_h   with   y = M_Eh x  a block-sparse contraction
    (32 positions per cluster).  y for all heads/both tables is computed with
    one small matmul per (128-position group, D-chunk): stationary = x tile,
    moving = a host-built block-diagonal EW/FW table, giving y TRANSPOSED
    (D on partitions) so the k_proj/v_proj matmuls need no transposes.
    This removes the full K/V GEMMs entirely (the dominant baseline cost).
  - the 3-kernel conv fusion over scores collapses to 5 "tap" matrices M_t in
    [P, P]: scores_conv[s] = sum_t q[s+t] @ (k_proj^T @ M_t); applied as 5
    PSUM-accumulated matmuls with a column-shifted (zero-padded) q^T operand.
  - adjacent heads are packed block-diagonally so every big matmul contracts
    over the full 128 partitions.
  - softmax has no max-subtraction (|scores| <~ 1.6, exp is safe in f32);
    Z = sum_c exp is computed by an all-ones block-diag matmul that also
    broadcasts Z to all 128 partitions, so normalization is one DVE op.
  - the attention phase is software-pipelined: the next unit's tap matmuls
    are emitted between a unit's taps and its Z/att matmuls so the PE never
    waits on the ACT exp.
  - fp8e4m3 DoubleRow matmuls double the contraction per instruction on the
    noise-tolerant scores path: q = wq^T x as 2 DR matmuls (wq, x fp8), the
    5 conv taps as 3 DR matmuls pairing (t0,t2),(t1,t3),(t4,zero) via a
    k-tile-stride-2 view of a single fp8 q plane.  Value path (K/V proj,
    attention-weighted sum, dense) stays bf16.
  - scheduling: taps(b, pr, n) reads a few columns past the 512-block
    boundary, so qt(b, pr, n+1) must be emitted ahead of it (paced fillers
    keep 4 slots of slack); input DMAs are few large descriptors (~650ns
    issue each) interleaved so the qt dc-chain chases the transfers.
"""
import sys
import numpy as np
import ml_dtypes

sys.path.insert(0, '/opt/trn_rl_repo')

B, S, D = 16, 2048, 512
H, P, C = 8, 64, 32
DEPTH = D // H           # 64
NCORES = 8
BLOC = B // NCORES       # 2 batches per core
NPAIR = H // 2           # 4 head pairs
SCH = 4                  # s-chunks of 512
SCW = S // SCH           # 512
NJ = S // 128            # 16 s-tiles of 128
NDC = D // 128           # 4 contraction chunks
NG = S // 128            # 16 position groups of 128 (4 clusters each)
QTW = S + 8              # padded fp8 q plane width
QS = 32.0                # fp8 scale for q
BS = 16.0                # fp8 scale for bdt (folded into bdm host-side)
XS = 32.0                # fp8 scale for x (qt path)
WQS = 16.0               # fp8 scale for wq

_CACHE = {}


def _build_nc():
    import concourse.tile as tile
    from concourse import mybir, bacc

    f32 = mybir.dt.float32
    bf16 = mybir.dt.bfloat16
    f8 = mybir.dt.float8e4
    DRM = mybir.MatmulPerfMode.DoubleRow

    # all parameters are host-pre-arranged partition-major so every input
    # DMA is a single contiguous descriptor (descriptor issue is ~650ns)
    nc = bacc.Bacc()
    xn = nc.declare_dram_parameter("xn", [BLOC, 4, 128, 4, D], bf16,
                                   isOutput=False)
    xT = nc.declare_dram_parameter("xT", [BLOC, 2, 128, 2, S], f8,
                                   isOutput=False)
    wq = nc.declare_dram_parameter("wq", [128, NPAIR, 2, 2, 128], f8,
                                   isOutput=False)
    wk = nc.declare_dram_parameter("wk", [128, NDC, D], bf16, isOutput=False)
    wv = nc.declare_dram_parameter("wv", [128, NDC, D], bf16, isOutput=False)
    dw = nc.declare_dram_parameter("dw", [128, NDC, D], bf16, isOutput=False)
    ewbd = nc.declare_dram_parameter("ewbd", [128, NG, 64], bf16,
                                     isOutput=False)
    bdm = nc.declare_dram_parameter("bdm", [128, 5, 128], bf16,
                                    isOutput=False)
    onesbd = nc.declare_dram_parameter("onesbd", [128, 128], bf16,
                                       isOutput=False)
    out = nc.declare_dram_parameter("out", [BLOC, S, D], bf16, isOutput=True)

    with tile.TileContext(nc) as tc:
        with tc.tile_pool(name="const", bufs=1) as cpool, \
             tc.tile_pool(name="big", bufs=1) as bigp, \
             tc.tile_pool(name="sm", bufs=4) as smp, \
             tc.tile_pool(name="bd", bufs=4) as bdp, \
             tc.tile_pool(name="ob", bufs=3) as obp, \
             tc.tile_pool(name="ps", bufs=1, space="PSUM") as psp:

            # ---- constants in SBUF (sync DMA queue; x uses gpsimd queue) ----
            ew_sb = cpool.tile([128, NG, 64], bf16)
            nc.sync.dma_start(out=ew_sb, in_=ewbd[:])
            wk_sb = cpool.tile([128, NDC, D], bf16)
            wv_sb = cpool.tile([128, NDC, D], bf16)
            wq_sb = cpool.tile([128, NPAIR, 2, 2, 128], f8)
            dw_sb = cpool.tile([128, NDC, D], bf16)
            for t_sb, t_dr in ((wk_sb, wk), (wv_sb, wv), (wq_sb, wq),
                               (dw_sb, dw)):
                nc.sync.dma_start(out=t_sb, in_=t_dr[:])
            bdm_sb = cpool.tile([128, 5, 128], bf16)
            nc.sync.dma_start(out=bdm_sb, in_=bdm[:])
            ones_sb = cpool.tile([128, 128], bf16)
            nc.sync.dma_start(out=ones_sb, in_=onesbd[:])

            st = [dict(expt={}) for _ in range(BLOC)]

            def emit_x_dma(b):
                # 6 single-descriptor DMAs per batch: xn in 4 chunks of 4
                # groups, xt in 2 chunks of 2 D-slices, interleaved so the
                # qt dc-chain can chase the transfers
                s = st[b]
                s["xnc"] = [bigp.tile([128, 4, D], bf16, tag="xnc", bufs=4,
                                      name=f"xn_{b}_{c}") for c in range(4)]
                s["xtc"] = [bigp.tile([128, 2, S], f8, tag="xt", bufs=4,
                                      name=f"xt_{b}_{h}") for h in range(2)]
                nc.sync.dma_start(out=s["xtc"][0], in_=xT[b, 0])
                nc.sync.dma_start(out=s["xtc"][1], in_=xT[b, 1])
                nc.gpsimd.dma_start(out=s["xnc"][0], in_=xn[b, 0])
                nc.gpsimd.dma_start(out=s["xnc"][1], in_=xn[b, 1])
                nc.gpsimd.dma_start(out=s["xnc"][2], in_=xn[b, 2])
                nc.gpsimd.dma_start(out=s["xnc"][3], in_=xn[b, 3])

            def xn_view(b, g, dc):
                return st[b]["xnc"][g // 4][:, g % 4,
                                            128 * dc:128 * (dc + 1)]

            def emit_yt(b, g):
                # yT[(dc), d, (t, h, c_lo) of group g] via stationary x tile;
                # all 4 dc outputs packed in one PSUM bank (single accum
                # group over disjoint column regions)
                s = st[b]
                if g == 0:
                    s["yt"] = bigp.tile([128, NDC, 2, H, NG, 4], bf16,
                                        tag="yt", bufs=2, name=f"yt_{b}")
                ps_y = psp.tile([128, 512], f32, tag="pssmall", bufs=2)
                for dc in range(NDC):
                    nc.tensor.matmul(
                        ps_y[:, 64 * dc:64 * (dc + 1)],
                        xn_view(b, g, dc),
                        ew_sb[:, g, :],
                        start=(dc == 0), stop=(dc == NDC - 1),
                        skip_group_check=True)
                for dc in range(NDC):
                    src = ps_y[:, 64 * dc:64 * (dc + 1)].rearrange(
                        "p (t h c) -> p t h c", t=2, h=H)
                    if g % 2 == 0:
                        nc.vector.tensor_copy(
                            out=s["yt"][:, dc, :, :, g, :], in_=src)
                    else:
                        nc.scalar.copy(
                            out=s["yt"][:, dc, :, :, g, :], in_=src)

            def emit_kpvp(b, pr):
                # kp/vp[c (2 heads), d (2 heads)] = yT^T @ Wk/Wv, block-diag;
                # both tables of the pair share one PSUM bank / accum group
                s = st[b]
                if pr == 0:
                    s["kp"] = bigp.tile([128, NPAIR, 128], bf16, tag="kpbd",
                                        bufs=2, name=f"kp_{b}")
                    s["vp"] = bigp.tile([128, NPAIR, 128], bf16, tag="vpbd",
                                        bufs=2, name=f"vp_{b}")
                    nc.vector.memset(s["kp"], 0.0)
                    nc.vector.memset(s["vp"], 0.0)
                ps = psp.tile([128, 512], f32, tag="pssmall", bufs=2)
                for te, w_sb in ((0, wk_sb), (1, wv_sb)):
                    for dc in range(NDC):
                        nc.tensor.matmul(
                            ps[:, 128 * te:128 * (te + 1)],
                            s["yt"][:, dc, te, 2 * pr:2 * pr + 2, :, :],
                            w_sb[:, dc, 128 * pr:128 * (pr + 1)],
                            start=(te == 0 and dc == 0),
                            stop=(te == 1 and dc == NDC - 1),
                            skip_group_check=True)
                for te, dst in ((0, s["kp"]), (1, s["vp"])):
                    o = 128 * te
                    if pr % 2 == 0:
                        nc.vector.tensor_copy(out=dst[0:64, pr, 0:64],
                                              in_=ps[0:64, o:o + 64])
                        nc.vector.tensor_copy(out=dst[64:128, pr, 64:128],
                                              in_=ps[64:128, o + 64:o + 128])
                    else:
                        nc.scalar.copy(out=dst[0:64, pr, 0:64],
                                       in_=ps[0:64, o:o + 64])
                        nc.scalar.copy(out=dst[64:128, pr, 64:128],
                                       in_=ps[64:128, o + 64:o + 128])

            def emit_qt(b, pr, n):
                # q stored as fp8e4m3 (x32) in two planes: plane1 is plane0
                # shifted left by one column, so DoubleRow tap pairs read
                # (q_shift_t, q_shift_{t+1}) with a clean plane stride.
                s = st[b]
                if pr == 0 and n == 0:
                    s["qt"] = bigp.tile([128, NPAIR, QTW], f8,
                                        tag="qT", bufs=2, name=f"qt_{b}")
                    nc.vector.memset(s["qt"][:, :, 0:2], 0.0)
                    nc.vector.memset(s["qt"][:, :, SCW * SCH + 2:], 0.0)
                # q = wq^T x via two fp8 DoubleRow matmuls (contraction 256
                # each: dc pair (2k, 2k+1) = one xt chunk)
                ps_q = psp.tile([128, SCW], f32, tag="psqd", bufs=2)
                for k in range(2):
                    nc.tensor.matmul(
                        ps_q,
                        wq_sb[:, pr, k, :, :],
                        s["xtc"][k][:, :, SCW * n:SCW * (n + 1)],
                        start=(k == 0), stop=(k == 1),
                        perf_mode=DRM)
                # both plane copies on ONE engine per psum tile (readers on
                # two engines race the next ring tenant's WAR), alternating
                # the engine between chunks for load balance
                qsc = QS / (WQS * XS)
                if (pr + n) % 2 == 0:
                    nc.vector.tensor_scalar_mul(
                        out=s["qt"][:, pr, 2 + SCW * n:2 + SCW * (n + 1)],
                        in0=ps_q, scalar1=qsc)
                else:
                    nc.scalar.mul(
                        s["qt"][:, pr, 2 + SCW * n:2 + SCW * (n + 1)],
                        ps_q, qsc)

            def emit_kt(b, pr):
                # bdt8[pair, {A,B}, 128] fp8 (x16 via host-scaled bdm):
                # pairs (t0,t1), (t2,t3), (t4, zero) for DoubleRow taps
                s = st[b]
                if pr == 0:
                    s["concat"] = bigp.tile([128, NPAIR, S], bf16,
                                            tag="concatT", bufs=2,
                                            name=f"concat_{b}")
                    s["bdt"] = {}
                bdt = bdp.tile([128, 3, 2, 128], f8, tag="bdt",
                               name=f"bdt_{b}_{pr}")
                s["bdt"][pr] = bdt
                ps4 = psp.tile([128, 512], f32, tag="pssmall", bufs=2)
                for t in range(4):
                    nc.tensor.matmul(ps4[:, 128 * t:128 * (t + 1)],
                                     s["kp"][:, pr, :], bdm_sb[:, t, :],
                                     start=(t == 0), stop=(t == 3),
                                     skip_group_check=True)
                nc.vector.tensor_copy(
                    out=bdt[:, 0:2, :, :],
                    in_=ps4.rearrange("q (i p m) -> q p i m", i=2, p=2))
                ps1 = psp.tile([128, 512], f32, tag="pssmall", bufs=2)
                nc.tensor.matmul(ps1[:, 0:128], s["kp"][:, pr, :],
                                 bdm_sb[:, 4, :], start=True, stop=True)
                nc.scalar.copy(out=bdt[:, 2, 0, :], in_=ps1[:, 0:128])
                nc.vector.memset(bdt[:, 2, 1, :], 0.0)

            def emit_taps(b, pr, n):
                s = st[b]
                bdt = s["bdt"][pr]
                import dataclasses
                ps_sc = psp.tile([128, SCW], f32, tag="pssc", bufs=2)
                for p, pb in enumerate((0, 1, 4)):  # pairs (0,2),(1,3),(4,-)
                    v = s["qt"][:, pr, SCW * n + pb:SCW * n + pb + SCW]
                    ap0 = [list(q) for q in v.ap]
                    v2 = dataclasses.replace(
                        v, ap=type(v.ap)([ap0[0], [2, 2], [1, SCW]]))
                    nc.tensor.matmul(
                        ps_sc,
                        bdt[:, p, :, :],
                        v2,
                        start=(p == 0), stop=(p == 2),
                        perf_mode=DRM)
                expt = smp.tile([128, SCW], bf16, tag="expt")
                nc.scalar.activation(
                    out=expt, in_=ps_sc,
                    func=mybir.ActivationFunctionType.Exp,
                    scale=1.0 / (BS * QS))
                s["expt"][(pr, n)] = expt

            def emit_zav(b, pr, n):
                s = st[b]
                expt = s["expt"].pop((pr, n))
                ps_z = psp.tile([128, SCW], f32, tag="psz", bufs=1)
                nc.tensor.matmul(ps_z, ones_sb, expt, start=True, stop=True)
                ps_at = psp.tile([128, SCW], f32, tag="psat", bufs=1)
                nc.tensor.matmul(ps_at, s["vp"][:, pr, :], expt,
                                 start=True, stop=True)
                # 1/Z: approx reciprocal (~18 bits, single DVE op)
                rzb = smp.tile([128, SCW], f32, tag="rzb")
                nc.vector.reciprocal_approx_fast(out=rzb, in_=ps_z)
                nc.vector.tensor_mul(
                    out=s["concat"][:, pr, SCW * n:SCW * (n + 1)],
                    in0=ps_at, in1=rzb)

            def emit_dense(b, j):
                # dense bias is added on the host after the gather; output
                # staged as bf16 (host upcasts) to halve copy + DMA cost
                s = st[b]
                ps_d = psp.tile([128, D], f32, tag="psqd", bufs=2)
                for dc in range(NDC):
                    nc.tensor.matmul(
                        ps_d,
                        s["concat"][:, dc, 128 * j:128 * (j + 1)],
                        dw_sb[:, dc, :],
                        start=(dc == 0), stop=(dc == NDC - 1))
                obuf = obp.tile([128, D], bf16, tag="obuf")
                nc.scalar.copy(out=obuf, in_=ps_d)
                nc.sync.dma_start(out=out[b, 128 * j:128 * (j + 1), :],
                                  in_=obuf)

            # ---- emission schedule ----
            # batch 0 pre-work; remaining qt(0) and batch-1 pre-work are
            # att-phase fillers, paced at one qt per attention unit so the
            # qt psum ring / plane-copy pipeline never backs up
            emit_x_dma(0)
            for g in range(NG):
                emit_yt(0, g)
                if g == 7:
                    emit_qt(0, 0, 0)
                elif g == 11:
                    emit_qt(0, 0, 1)
                elif g == 15:
                    emit_qt(0, 0, 2)
            emit_qt(0, 0, 3)
            for pr in range(NPAIR):
                emit_kpvp(0, pr)
            emit_x_dma(1)
            # qt fillers: qt(0, pr1-3) pr-major, then qt(1, *) n-major
            # (matches the n-outer att(1) consumption order)
            qts = [(0, pr, n) for pr in range(1, NPAIR)
                   for n in range(SCH)] + \
                  [(1, pr, n) for n in range(SCH) for pr in range(NPAIR)]
            others = [(emit_yt, (1, g)) for g in range(NG)] + \
                     [(emit_kpvp, (1, pr)) for pr in range(NPAIR)] + \
                     [(emit_kt, (1, pr)) for pr in range(NPAIR)]
            qi = oi = 0
            pend = None
            for pr in range(NPAIR):
                emit_kt(0, pr)
                for n in range(SCH):
                    # qt fillers lead their consumers: taps(b, pr, n) reads
                    # a few columns past the block boundary, so it needs
                    # qt(b, pr, n+1) as well -- keep >= 3 slots of slack by
                    # draining two qt per slot in the last att(0) pairs
                    slot = 4 * pr + n
                    for _ in range(2 if slot >= 12 else 1):
                        if qi < len(qts):
                            emit_qt(*qts[qi]); qi += 1
                    emit_taps(0, pr, n)
                    if pend is not None:
                        emit_zav(0, *pend)
                    pend = (pr, n)
                    for _ in range(2 if oi % 3 != 2 else 1):
                        if oi < len(others):
                            f, a = others[oi]; f(*a); oi += 1
            emit_zav(0, *pend)
            while oi < len(others):
                f, a = others[oi]; f(*a); oi += 1
            # att(1) (n-outer) with remaining qt(1) + batch-0/1 dense fillers
            pend = None
            dj0 = 0
            for n in range(SCH):
                for pr in range(NPAIR):
                    if qi < len(qts):
                        emit_qt(*qts[qi]); qi += 1
                    emit_taps(1, pr, n)
                    if pend is not None:
                        emit_zav(1, *pend)
                    pend = (pr, n)
                    if dj0 < NJ:
                        emit_dense(0, dj0); dj0 += 1
                if n > 0:
                    for j in range(SCH * (n - 1), SCH * n):
                        emit_dense(1, j)
            emit_zav(1, *pend)
            while dj0 < NJ:
                emit_dense(0, dj0); dj0 += 1
            for j in range(SCH * (SCH - 1), NJ):
                emit_dense(1, j)

    nc.finalize()
    return nc


def _prep_inputs(x, mask, wq, wk, wv, EW, FW, conv_w1, conv_w3, conv_w5,
                 conv_b, dense_w, dense_b, cluster_table):
    """Host-side restructuring -> per-core input maps."""
    bf = ml_dtypes.bfloat16
    x = np.ascontiguousarray(np.asarray(x, np.float32))
    mask = np.asarray(mask)
    counts = np.clip(mask.astype(np.int64).sum(1), 1, S)
    pos = np.asarray(cluster_table)[counts - 1]          # [B, P, C]
    if not (pos == pos[0]).all():
        raise NotImplementedError("per-batch cluster tables not supported")
    p0 = pos[0]                                          # [P, C]
    expect = 32 * np.arange(P)[:, None] + np.arange(C)[None, :]
    if not np.array_equal(p0, expect):
        raise NotImplementedError("non-contiguous cluster layout")

    scale = 1.0 / np.sqrt(np.float32(DEPTH))
    # block-diagonal EW/FW table: EWBD[g][c_lo*32 + l, t*32 + h*4 + c_lo]
    EWn = np.asarray(EW, np.float32) * scale             # [H, P, C]
    FWn = np.asarray(FW, np.float32)
    EWBD = np.zeros((NG, 128, 64), np.float32)
    for c_lo in range(4):
        rows = slice(c_lo * 32, (c_lo + 1) * 32)
        for h in range(H):
            EWBD[:, rows, 0 * 32 + h * 4 + c_lo] = EWn[h, c_lo::4, :]
            EWBD[:, rows, 1 * 32 + h * 4 + c_lo] = FWn[h, c_lo::4, :]

    # conv -> 5 tap matrices
    wp = np.arange(P)[:, None]
    jj = np.arange(P)[None, :]
    ii = wp - jj + 31
    valid = (ii >= 0) & (ii < P)
    ii = np.clip(ii, 0, P - 1)
    M = {t: np.zeros((P, P), np.float32) for t in range(-2, 3)}
    for cw, hk in ((conv_w1, 1), (conv_w3, 3), (conv_w5, 5)):
        cw = np.asarray(cw, np.float32)
        pad = (hk - 1) // 2
        for dy in range(hk):
            filt = cw[dy, :, 0, 0]
            M[dy - pad] += np.where(valid, filt[ii], 0.0) / 3.0
    # BS folded in: bdt comes out of the kt matmuls pre-scaled for fp8
    BDM = np.zeros((5, 128, 128), np.float32)
    for ti in range(5):
        BDM[ti, :64, :64] = M[ti - 2] * BS
        BDM[ti, 64:, 64:] = M[ti - 2] * BS
    bbar = float(np.asarray(conv_b, np.float32).mean())
    if abs(bbar) > 1e-30:
        raise NotImplementedError("nonzero conv bias not folded")

    ones_bd = np.zeros((128, 128), np.float32)
    ones_bd[:64, :64] = 1.0
    ones_bd[64:, 64:] = 1.0

    # shard x (both layouts), partition-major for single-descriptor DMAs
    f8 = ml_dtypes.float8_e4m3fn
    xsh = x.reshape(NCORES, BLOC, S, D)
    pmaj = lambda w: np.ascontiguousarray(
        np.asarray(w, np.float32).reshape(NDC, 128, D).transpose(1, 0, 2)
    ).astype(bf)
    # wq8[p, pr, k, l, m] = wq[(2k+l)*128 + p, 128*pr + m] * WQS (fp8)
    wqf = np.asarray(wq, np.float32) * WQS
    wq8 = np.ascontiguousarray(
        wqf.reshape(2, 2, 128, NPAIR, 128).transpose(2, 3, 0, 1, 4)
    ).astype(f8)
    in_maps = []
    shared = dict(
        wq=wq8, wk=pmaj(wk), wv=pmaj(wv), dw=pmaj(dense_w),
        ewbd=np.ascontiguousarray(EWBD.transpose(1, 0, 2)).astype(bf),
        bdm=np.ascontiguousarray(BDM.transpose(1, 0, 2)).astype(bf),
        onesbd=ones_bd.astype(bf),
    )
    for c in range(NCORES):
        m = dict(shared)
        # xn chunks: [4][128, 4, D]: chunk cc, partition p, group g_lo
        m["xn"] = np.ascontiguousarray(
            xsh[c].reshape(BLOC, 4, 4, 128, D).transpose(0, 1, 3, 2, 4)
        ).astype(bf)
        # xt chunks: [2][128, 2, S] fp8 (x XS): chunk h, partition p, dc_lo
        xt = (xsh[c] * XS).transpose(0, 2, 1).reshape(BLOC, 2, 2, 128, S)
        m["xT"] = np.ascontiguousarray(xt.transpose(0, 1, 3, 2, 4)).astype(f8)
        in_maps.append(m)
    return in_maps


def _run(in_maps, trace=False, tmpdir=None):
    from concourse.bass_utils import run_bass_kernel_spmd
    if "nc" not in _CACHE:
        _CACHE["nc"] = _build_nc()
    kw = {}
    if trace:
        _install_ntff_hook()
        kw = dict(trace=True, tmpdir=tmpdir)
    return run_bass_kernel_spmd(_CACHE["nc"], in_maps,
                                core_ids=list(range(NCORES)), **kw)


def _install_ntff_hook():
    import types, importlib.util as ilu
    if "antenv.axon_hooks" in sys.modules:
        return
    spec = ilu.spec_from_file_location(
        "trn_boot_mod", "/root/.axon_site/trn_agent_boot/trn_boot.py")
    tb = ilu.module_from_spec(spec)
    spec.loader.exec_module(tb)
    hook = tb._ntff_profile_via_ctypes("/opt/axon/libaxon_pjrt.so")
    mod = types.ModuleType("antenv.axon_hooks")
    mod.get_axon_ntff_profile_hook = lambda: hook
    import antenv  # noqa: F401
    sys.modules["antenv.axon_hooks"] = mod


def kernel(**inputs) -> np.ndarray:
    in_maps = _prep_inputs(**inputs)
    r = _run(in_maps)
    res = np.concatenate([r.results[c]["out"].astype(np.float32)
                          for c in range(NCORES)], axis=0)
    res += np.asarray(inputs["dense_b"], np.float32)[None, None, :]
    return res


# revision 72
# speedup vs baseline: 1.0136x; 1.0136x over previous
"""Clustered Linformer Attention — Trainium2 Bass kernel, 8 NeuronCores.

Strategy: data-parallel over batch (2 batches/core, no collectives).
~142us HW exec (baseline 254us), rel err ~1.56e-2 (gate 2e-2, deterministic).
Math restructuring (exact vs reference up to rounding):
  - mask is all-ones => cluster c holds positions [32c, 32c+32); K/V are only
    consumed through the cluster projections, so reassociate:
      k_proj_h = (M_Eh x) Wk_h   with   y = M_Eh x  a block-sparse contraction
    (32 positions per cluster).  y for all heads/both tables is computed with
    one small matmul per (128-position group, D-chunk): stationary = x tile,
    moving = a host-built block-diagonal EW/FW table, giving y TRANSPOSED
    (D on partitions) so the k_proj/v_proj matmuls need no transposes.
    This removes the full K/V GEMMs entirely (the dominant baseline cost).
  - the 3-kernel conv fusion over scores collapses to 5 "tap" matrices M_t in
    [P, P]: scores_conv[s] = sum_t q[s+t] @ (k_proj^T @ M_t); applied as 5
    PSUM-accumulated matmuls with a column-shifted (zero-padded) q^T operand.
  - adjacent heads are packed block-diagonally so every big matmul contracts
    over the full 128 partitions.
  - softmax has no max-subtraction (|scores| <~ 1.6, exp is safe in f32);
    Z = sum_c exp is computed by an all-ones block-diag matmul that also
    broadcasts Z to all 128 partitions, so normalization is one DVE op.
  - the attention phase is software-pipelined: the next unit's tap matmuls
    are emitted between a unit's taps and its Z/att matmuls so the PE never
    waits on the ACT exp.
  - fp8e4m3 DoubleRow matmuls double the contraction per instruction on the
    noise-tolerant scores path: q = wq^T x as 2 DR matmuls (wq, x fp8), the
    5 conv taps as 3 DR matmuls pairing (t0,t2),(t1,t3),(t4,zero) via a
    k-tile-stride-2 view of a single fp8 q plane.  Value path (K/V proj,
    attention-weighted sum, dense) stays bf16.
  - scheduling: taps(b, pr, n) reads a few columns past the 512-block
    boundary, so qt(b, pr, n+1) must be emitted ahead of it (paced fillers
    keep 4 slots of slack); input DMAs are few large descriptors (~650ns
    issue each) interleaved so the qt dc-chain chases the transfers.
"""
import sys
import numpy as np
import ml_dtypes

sys.path.insert(0, '/opt/trn_rl_repo')

B, S, D = 16, 2048, 512
H, P, C = 8, 64, 32
DEPTH = D // H           # 64
NCORES = 8
BLOC = B // NCORES       # 2 batches per core
NPAIR = H // 2           # 4 head pairs
SCH = 4                  # s-chunks of 512
SCW = S // SCH           # 512
NJ = S // 128            # 16 s-tiles of 128
NDC = D // 128           # 4 contraction chunks
NG = S // 128            # 16 position groups of 128 (4 clusters each)
QTW = S + 8              # padded fp8 q plane width
QS = 32.0                # fp8 scale for q
BS = 16.0                # fp8 scale for bdt (folded into bdm host-side)
XS = 32.0                # fp8 scale for x (qt path)
WQS = 16.0               # fp8 scale for wq

_CACHE = {}


def _build_nc():
    import concourse.tile as tile
    from concourse import mybir, bacc

    f32 = mybir.dt.float32
    bf16 = mybir.dt.bfloat16
    f8 = mybir.dt.float8e4
    DRM = mybir.MatmulPerfMode.DoubleRow

    # all parameters are host-pre-arranged partition-major so every input
    # DMA is a single contiguous descriptor (descriptor issue is ~650ns)
    nc = bacc.Bacc()
    xn = nc.declare_dram_parameter("xn", [BLOC, 4, 128, 4, D], bf16,
                                   isOutput=False)
    xT = nc.declare_dram_parameter("xT", [BLOC, 2, 128, 2, S], f8,
                                   isOutput=False)
    wq = nc.declare_dram_parameter("wq", [128, NPAIR, 2, 2, 128], f8,
                                   isOutput=False)
    wk = nc.declare_dram_parameter("wk", [128, NDC, D], bf16, isOutput=False)
    wv = nc.declare_dram_parameter("wv", [128, NDC, D], bf16, isOutput=False)
    dw = nc.declare_dram_parameter("dw", [128, NDC, D], bf16, isOutput=False)
    ewbd = nc.declare_dram_parameter("ewbd", [128, NG, 64], bf16,
                                     isOutput=False)
    bdm = nc.declare_dram_parameter("bdm", [128, 5, 128], bf16,
                                    isOutput=False)
    onesbd = nc.declare_dram_parameter("onesbd", [128, 128], bf16,
                                       isOutput=False)
    out = nc.declare_dram_parameter("out", [BLOC, S, D], bf16, isOutput=True)

    with tile.TileContext(nc) as tc:
        with tc.tile_pool(name="const", bufs=1) as cpool, \
             tc.tile_pool(name="big", bufs=1) as bigp, \
             tc.tile_pool(name="sm", bufs=4) as smp, \
             tc.tile_pool(name="bd", bufs=4) as bdp, \
             tc.tile_pool(name="ob", bufs=3) as obp, \
             tc.tile_pool(name="ps", bufs=1, space="PSUM") as psp:

            # ---- constants in SBUF (sync DMA queue; x uses gpsimd queue) ----
            ew_sb = cpool.tile([128, NG, 64], bf16)
            nc.sync.dma_start(out=ew_sb, in_=ewbd[:])
            wk_sb = cpool.tile([128, NDC, D], bf16)
            wv_sb = cpool.tile([128, NDC, D], bf16)
            wq_sb = cpool.tile([128, NPAIR, 2, 2, 128], f8)
            dw_sb = cpool.tile([128, NDC, D], bf16)
            for t_sb, t_dr in ((wk_sb, wk), (wv_sb, wv), (wq_sb, wq),
                               (dw_sb, dw)):
                nc.sync.dma_start(out=t_sb, in_=t_dr[:])
            bdm_sb = cpool.tile([128, 5, 128], bf16)
            nc.sync.dma_start(out=bdm_sb, in_=bdm[:])
            ones_sb = cpool.tile([128, 128], bf16)
            nc.sync.dma_start(out=ones_sb, in_=onesbd[:])

            st = [dict(expt={}) for _ in range(BLOC)]

            def emit_x_dma(b):
                # 6 single-descriptor DMAs per batch: xn in 4 chunks of 4
                # groups, xt in 2 chunks of 2 D-slices, interleaved so the
                # qt dc-chain can chase the transfers
                s = st[b]
                s["xnc"] = [bigp.tile([128, 4, D], bf16, tag="xnc", bufs=4,
                                      name=f"xn_{b}_{c}") for c in range(4)]
                s["xtc"] = [bigp.tile([128, 2, S], f8, tag="xt", bufs=4,
                                      name=f"xt_{b}_{h}") for h in range(2)]
                nc.gpsimd.dma_start(out=s["xnc"][0], in_=xn[b, 0])
                nc.gpsimd.dma_start(out=s["xnc"][1], in_=xn[b, 1])
                nc.gpsimd.dma_start(out=s["xtc"][0], in_=xT[b, 0])
                nc.gpsimd.dma_start(out=s["xnc"][2], in_=xn[b, 2])
                nc.gpsimd.dma_start(out=s["xnc"][3], in_=xn[b, 3])
                nc.gpsimd.dma_start(out=s["xtc"][1], in_=xT[b, 1])

            def xn_view(b, g, dc):
                return st[b]["xnc"][g // 4][:, g % 4,
                                            128 * dc:128 * (dc + 1)]

            def emit_yt(b, g):
                # yT[(dc), d, (t, h, c_lo) of group g] via stationary x tile;
                # all 4 dc outputs packed in one PSUM bank (single accum
                # group over disjoint column regions)
                s = st[b]
                if g == 0:
                    s["yt"] = bigp.tile([128, NDC, 2, H, NG, 4], bf16,
                                        tag="yt", bufs=2, name=f"yt_{b}")
                ps_y = psp.tile([128, 512], f32, tag="pssmall", bufs=2)
                for dc in range(NDC):
                    nc.tensor.matmul(
                        ps_y[:, 64 * dc:64 * (dc + 1)],
                        xn_view(b, g, dc),
                        ew_sb[:, g, :],
                        start=(dc == 0), stop=(dc == NDC - 1),
                        skip_group_check=True)
                for dc in range(NDC):
                    src = ps_y[:, 64 * dc:64 * (dc + 1)].rearrange(
                        "p (t h c) -> p t h c", t=2, h=H)
                    if g % 2 == 0:
                        nc.vector.tensor_copy(
                            out=s["yt"][:, dc, :, :, g, :], in_=src)
                    else:
                        nc.scalar.copy(
                            out=s["yt"][:, dc, :, :, g, :], in_=src)

            def emit_kpvp(b, pr):
                # kp/vp[c (2 heads), d (2 heads)] = yT^T @ Wk/Wv, block-diag;
                # both tables of the pair share one PSUM bank / accum group
                s = st[b]
                if pr == 0:
                    s["kp"] = bigp.tile([128, NPAIR, 128], bf16, tag="kpbd",
                                        bufs=2, name=f"kp_{b}")
                    s["vp"] = bigp.tile([128, NPAIR, 128], bf16, tag="vpbd",
                                        bufs=2, name=f"vp_{b}")
                    nc.vector.memset(s["kp"], 0.0)
                    nc.vector.memset(s["vp"], 0.0)
                ps = psp.tile([128, 512], f32, tag="pssmall", bufs=2)
                for te, w_sb in ((0, wk_sb), (1, wv_sb)):
                    for dc in range(NDC):
                        nc.tensor.matmul(
                            ps[:, 128 * te:128 * (te + 1)],
                            s["yt"][:, dc, te, 2 * pr:2 * pr + 2, :, :],
                            w_sb[:, dc, 128 * pr:128 * (pr + 1)],
                            start=(te == 0 and dc == 0),
                            stop=(te == 1 and dc == NDC - 1),
                            skip_group_check=True)
                for te, dst in ((0, s["kp"]), (1, s["vp"])):
                    o = 128 * te
                    if pr % 2 == 0:
                        nc.vector.tensor_copy(out=dst[0:64, pr, 0:64],
                                              in_=ps[0:64, o:o + 64])
                        nc.vector.tensor_copy(out=dst[64:128, pr, 64:128],
                                              in_=ps[64:128, o + 64:o + 128])
                    else:
                        nc.scalar.copy(out=dst[0:64, pr, 0:64],
                                       in_=ps[0:64, o:o + 64])
                        nc.scalar.copy(out=dst[64:128, pr, 64:128],
                                       in_=ps[64:128, o + 64:o + 128])

            def emit_qt(b, pr, n):
                # q stored as fp8e4m3 (x32) in two planes: plane1 is plane0
                # shifted left by one column, so DoubleRow tap pairs read
                # (q_shift_t, q_shift_{t+1}) with a clean plane stride.
                s = st[b]
                if pr == 0 and n == 0:
                    s["qt"] = bigp.tile([128, NPAIR, QTW], f8,
                                        tag="qT", bufs=2, name=f"qt_{b}")
                    nc.vector.memset(s["qt"][:, :, 0:2], 0.0)
                    nc.vector.memset(s["qt"][:, :, SCW * SCH + 2:], 0.0)
                # q = wq^T x via two fp8 DoubleRow matmuls (contraction 256
                # each: dc pair (2k, 2k+1) = one xt chunk)
                ps_q = psp.tile([128, SCW], f32, tag="psqd", bufs=2)
                for k in range(2):
                    nc.tensor.matmul(
                        ps_q,
                        wq_sb[:, pr, k, :, :],
                        s["xtc"][k][:, :, SCW * n:SCW * (n + 1)],
                        start=(k == 0), stop=(k == 1),
                        perf_mode=DRM)
                # both plane copies on ONE engine per psum tile (readers on
                # two engines race the next ring tenant's WAR), alternating
                # the engine between chunks for load balance
                qsc = QS / (WQS * XS)
                if (pr + n) % 2 == 0:
                    nc.vector.tensor_scalar_mul(
                        out=s["qt"][:, pr, 2 + SCW * n:2 + SCW * (n + 1)],
                        in0=ps_q, scalar1=qsc)
                else:
                    nc.scalar.mul(
                        s["qt"][:, pr, 2 + SCW * n:2 + SCW * (n + 1)],
                        ps_q, qsc)

            def emit_kt(b, pr):
                # bdt8[pair, {A,B}, 128] fp8 (x16 via host-scaled bdm):
                # pairs (t0,t1), (t2,t3), (t4, zero) for DoubleRow taps
                s = st[b]
                if pr == 0:
                    s["concat"] = bigp.tile([128, NPAIR, S], bf16,
                                            tag="concatT", bufs=2,
                                            name=f"concat_{b}")
                    s["bdt"] = {}
                bdt = bdp.tile([128, 3, 2, 128], f8, tag="bdt",
                               name=f"bdt_{b}_{pr}")
                s["bdt"][pr] = bdt
                ps4 = psp.tile([128, 512], f32, tag="pssmall", bufs=2)
                for t in range(4):
                    nc.tensor.matmul(ps4[:, 128 * t:128 * (t + 1)],
                                     s["kp"][:, pr, :], bdm_sb[:, t, :],
                                     start=(t == 0), stop=(t == 3),
                                     skip_group_check=True)
                nc.vector.tensor_copy(
                    out=bdt[:, 0:2, :, :],
                    in_=ps4.rearrange("q (i p m) -> q p i m", i=2, p=2))
                ps1 = psp.tile([128, 512], f32, tag="pssmall", bufs=2)
                nc.tensor.matmul(ps1[:, 0:128], s["kp"][:, pr, :],
                                 bdm_sb[:, 4, :], start=True, stop=True)
                nc.scalar.copy(out=bdt[:, 2, 0, :], in_=ps1[:, 0:128])
                nc.vector.memset(bdt[:, 2, 1, :], 0.0)

            def emit_taps(b, pr, n):
                s = st[b]
                bdt = s["bdt"][pr]
                import dataclasses
                ps_sc = psp.tile([128, SCW], f32, tag="pssc", bufs=2)
                for p, pb in enumerate((0, 1, 4)):  # pairs (0,2),(1,3),(4,-)
                    v = s["qt"][:, pr, SCW * n + pb:SCW * n + pb + SCW]
                    ap0 = [list(q) for q in v.ap]
                    v2 = dataclasses.replace(
                        v, ap=type(v.ap)([ap0[0], [2, 2], [1, SCW]]))
                    nc.tensor.matmul(
                        ps_sc,
                        bdt[:, p, :, :],
                        v2,
                        start=(p == 0), stop=(p == 2),
                        perf_mode=DRM)
                expt = smp.tile([128, SCW], bf16, tag="expt")
                nc.scalar.activation(
                    out=expt, in_=ps_sc,
                    func=mybir.ActivationFunctionType.Exp,
                    scale=1.0 / (BS * QS))
                s["expt"][(pr, n)] = expt

            def emit_zav(b, pr, n):
                s = st[b]
                expt = s["expt"].pop((pr, n))
                ps_z = psp.tile([128, SCW], f32, tag="psz", bufs=1)
                nc.tensor.matmul(ps_z, ones_sb, expt, start=True, stop=True)
                ps_at = psp.tile([128, SCW], f32, tag="psat", bufs=1)
                nc.tensor.matmul(ps_at, s["vp"][:, pr, :], expt,
                                 start=True, stop=True)
                # 1/Z: approx reciprocal (~18 bits, single DVE op)
                rzb = smp.tile([128, SCW], f32, tag="rzb")
                nc.vector.reciprocal_approx_fast(out=rzb, in_=ps_z)
                nc.vector.tensor_mul(
                    out=s["concat"][:, pr, SCW * n:SCW * (n + 1)],
                    in0=ps_at, in1=rzb)

            def emit_dense(b, j):
                # dense bias is added on the host after the gather; output
                # staged as bf16 (host upcasts) to halve copy + DMA cost
                s = st[b]
                ps_d = psp.tile([128, D], f32, tag="psqd", bufs=2)
                for dc in range(NDC):
                    nc.tensor.matmul(
                        ps_d,
                        s["concat"][:, dc, 128 * j:128 * (j + 1)],
                        dw_sb[:, dc, :],
                        start=(dc == 0), stop=(dc == NDC - 1))
                obuf = obp.tile([128, D], bf16, tag="obuf")
                nc.scalar.copy(out=obuf, in_=ps_d)
                nc.sync.dma_start(out=out[b, 128 * j:128 * (j + 1), :],
                                  in_=obuf)

            # ---- emission schedule ----
            # batch 0 pre-work; remaining qt(0) and batch-1 pre-work are
            # att-phase fillers, paced at one qt per attention unit so the
            # qt psum ring / plane-copy pipeline never backs up
            emit_x_dma(0)
            for g in range(NG):
                emit_yt(0, g)
            for pr in range(NPAIR):
                emit_kpvp(0, pr)
            for n in range(SCH):
                emit_qt(0, 0, n)
            emit_x_dma(1)
            # qt fillers: qt(0, pr1-3) pr-major, then qt(1, *) n-major
            # (matches the n-outer att(1) consumption order)
            qts = [(0, pr, n) for pr in range(1, NPAIR)
                   for n in range(SCH)] + \
                  [(1, pr, n) for n in range(SCH) for pr in range(NPAIR)]
            others = [(emit_yt, (1, g)) for g in range(NG)] + \
                     [(emit_kpvp, (1, pr)) for pr in range(NPAIR)] + \
                     [(emit_kt, (1, pr)) for pr in range(NPAIR)]
            qi = oi = 0
            pend = None
            for pr in range(NPAIR):
                emit_kt(0, pr)
                for n in range(SCH):
                    # qt fillers lead their consumers: taps(b, pr, n) reads
                    # a few columns past the block boundary, so it needs
                    # qt(b, pr, n+1) as well -- keep >= 3 slots of slack by
                    # draining two qt per slot in the last att(0) pairs
                    slot = 4 * pr + n
                    for _ in range(2 if slot >= 12 else 1):
                        if qi < len(qts):
                            emit_qt(*qts[qi]); qi += 1
                    emit_taps(0, pr, n)
                    if pend is not None:
                        emit_zav(0, *pend)
                    pend = (pr, n)
                    for _ in range(2 if oi % 3 != 2 else 1):
                        if oi < len(others):
                            f, a = others[oi]; f(*a); oi += 1
            emit_zav(0, *pend)
            while oi < len(others):
                f, a = others[oi]; f(*a); oi += 1
            # att(1) (n-outer) with remaining qt(1) + batch-0/1 dense fillers
            pend = None
            dj0 = 0
            for n in range(SCH):
                for pr in range(NPAIR):
                    if qi < len(qts):
                        emit_qt(*qts[qi]); qi += 1
                    emit_taps(1, pr, n)
                    if pend is not None:
                        emit_zav(1, *pend)
                    pend = (pr, n)
                    if dj0 < NJ:
                        emit_dense(0, dj0); dj0 += 1
                if n > 0:
                    for j in range(SCH * (n - 1), SCH * n):
                        emit_dense(1, j)
            emit_zav(1, *pend)
            while dj0 < NJ:
                emit_dense(0, dj0); dj0 += 1
            for j in range(SCH * (SCH - 1), NJ):
                emit_dense(1, j)

    nc.finalize()
    return nc


def _prep_inputs(x, mask, wq, wk, wv, EW, FW, conv_w1, conv_w3, conv_w5,
                 conv_b, dense_w, dense_b, cluster_table):
    """Host-side restructuring -> per-core input maps."""
    bf = ml_dtypes.bfloat16
    x = np.ascontiguousarray(np.asarray(x, np.float32))
    mask = np.asarray(mask)
    counts = np.clip(mask.astype(np.int64).sum(1), 1, S)
    pos = np.asarray(cluster_table)[counts - 1]          # [B, P, C]
    if not (pos == pos[0]).all():
        raise NotImplementedError("per-batch cluster tables not supported")
    p0 = pos[0]                                          # [P, C]
    expect = 32 * np.arange(P)[:, None] + np.arange(C)[None, :]
    if not np.array_equal(p0, expect):
        raise NotImplementedError("non-contiguous cluster layout")

    scale = 1.0 / np.sqrt(np.float32(DEPTH))
    # block-diagonal EW/FW table: EWBD[g][c_lo*32 + l, t*32 + h*4 + c_lo]
    EWn = np.asarray(EW, np.float32) * scale             # [H, P, C]
    FWn = np.asarray(FW, np.float32)
    EWBD = np.zeros((NG, 128, 64), np.float32)
    for c_lo in range(4):
        rows = slice(c_lo * 32, (c_lo + 1) * 32)
        for h in range(H):
            EWBD[:, rows, 0 * 32 + h * 4 + c_lo] = EWn[h, c_lo::4, :]
            EWBD[:, rows, 1 * 32 + h * 4 + c_lo] = FWn[h, c_lo::4, :]

    # conv -> 5 tap matrices
    wp = np.arange(P)[:, None]
    jj = np.arange(P)[None, :]
    ii = wp - jj + 31
    valid = (ii >= 0) & (ii < P)
    ii = np.clip(ii, 0, P - 1)
    M = {t: np.zeros((P, P), np.float32) for t in range(-2, 3)}
    for cw, hk in ((conv_w1, 1), (conv_w3, 3), (conv_w5, 5)):
        cw = np.asarray(cw, np.float32)
        pad = (hk - 1) // 2
        for dy in range(hk):
            filt = cw[dy, :, 0, 0]
            M[dy - pad] += np.where(valid, filt[ii], 0.0) / 3.0
    # BS folded in: bdt comes out of the kt matmuls pre-scaled for fp8
    BDM = np.zeros((5, 128, 128), np.float32)
    for ti in range(5):
        BDM[ti, :64, :64] = M[ti - 2] * BS
        BDM[ti, 64:, 64:] = M[ti - 2] * BS
    bbar = float(np.asarray(conv_b, np.float32).mean())
    if abs(bbar) > 1e-30:
        raise NotImplementedError("nonzero conv bias not folded")

    ones_bd = np.zeros((128, 128), np.float32)
    ones_bd[:64, :64] = 1.0
    ones_bd[64:, 64:] = 1.0

    # shard x (both layouts), partition-major for single-descriptor DMAs
    f8 = ml_dtypes.float8_e4m3fn
    xsh = x.reshape(NCORES, BLOC, S, D)
    pmaj = lambda w: np.ascontiguousarray(
        np.asarray(w, np.float32).reshape(NDC, 128, D).transpose(1, 0, 2)
    ).astype(bf)
    # wq8[p, pr, k, l, m] = wq[(2k+l)*128 + p, 128*pr + m] * WQS (fp8)
    wqf = np.asarray(wq, np.float32) * WQS
    wq8 = np.ascontiguousarray(
        wqf.reshape(2, 2, 128, NPAIR, 128).transpose(2, 3, 0, 1, 4)
    ).astype(f8)
    in_maps = []
    shared = dict(
        wq=wq8, wk=pmaj(wk), wv=pmaj(wv), dw=pmaj(dense_w),
        ewbd=np.ascontiguousarray(EWBD.transpose(1, 0, 2)).astype(bf),
        bdm=np.ascontiguousarray(BDM.transpose(1, 0, 2)).astype(bf),
        onesbd=ones_bd.astype(bf),
    )
    for c in range(NCORES):
        m = dict(shared)
        # xn chunks: [4][128, 4, D]: chunk cc, partition p, group g_lo
        m["xn"] = np.ascontiguousarray(
            xsh[c].reshape(BLOC, 4, 4, 128, D).transpose(0, 1, 3, 2, 4)
        ).astype(bf)
        # xt chunks: [2][128, 2, S] fp8 (x XS): chunk h, partition p, dc_lo
        xt = (xsh[c] * XS).transpose(0, 2, 1).reshape(BLOC, 2, 2, 128, S)
        m["xT"] = np.ascontiguousarray(xt.transpose(0, 1, 3, 2, 4)).astype(f8)
        in_maps.append(m)
    return in_maps


def _run(in_maps, trace=False, tmpdir=None):
    from concourse.bass_utils import run_bass_kernel_spmd
    if "nc" not in _CACHE:
        _CACHE["nc"] = _build_nc()
    kw = {}
    if trace:
        _install_ntff_hook()
        kw = dict(trace=True, tmpdir=tmpdir)
    return run_bass_kernel_spmd(_CACHE["nc"], in_maps,
                                core_ids=list(range(NCORES)), **kw)


def _install_ntff_hook():
    import types, importlib.util as ilu
    if "antenv.axon_hooks" in sys.modules:
        return
    spec = ilu.spec_from_file_location(
        "trn_boot_mod", "/root/.axon_site/trn_agent_boot/trn_boot.py")
    tb = ilu.module_from_spec(spec)
    spec.loader.exec_module(tb)
    hook = tb._ntff_profile_via_ctypes("/opt/axon/libaxon_pjrt.so")
    mod = types.ModuleType("antenv.axon_hooks")
    mod.get_axon_ntff_profile_hook = lambda: hook
    import antenv  # noqa: F401
    sys.modules["antenv.axon_hooks"] = mod


def kernel(**inputs) -> np.ndarray:
    in_maps = _prep_inputs(**inputs)
    r = _run(in_maps)
    res = np.concatenate([r.results[c]["out"].astype(np.float32)
                          for c in range(NCORES)], axis=0)
    res += np.asarray(inputs["dense_b"], np.float32)[None, None, :]
    return res
